# revision 1
# baseline (speedup 1.0000x reference)
"""Trainium2 Bass kernel for the CSTR (evaporator) 1M-step scan.

Parallel-in-time: the per-step map is contractive (slow mode ~0.9665/step),
so the trajectory is split into 1024 segments (8 cores x 128 lanes) of
L=1024 steps, each extended K=192 steps back ("spin-up") so an arbitrary
segment-entry state converges below tolerance before the graded region.
Within each lane's window the nonlinear recurrence

  x0' = x0*(SC(u0) - c02*x0 - c03*x1) + c01
  x1' = SA(u1)*x1 + a10*x0 + SB(u0,u1)

is solved by 2 Picard-Gauss-Seidel sweeps (second sweep re-scans from
column 64). States are rescaled (Y0 = x0/c01, Y1 = x1/(a10*c01)) so the
Y0-scan additive term is the constant 1.0 and the Y1-scan additive term
becomes c = Y0 + SBpa + rec_n, which the TENSOR engine accumulates in
PSUM via identity matmuls (fp32, exact) — the Y1 scans read their data1
operand directly from PSUM. The vector engine runs only the 4 linear
scans (tensor_tensor_scan), the reciprocal, and the sweep-2 coefficient
links; all affine precompute runs on the scalar (ACT) engine. Everything
is pipelined in column chunks (PSUM accumulation pieces never cross the
512-col bank boundary). Input u is staged in fp16 (accuracy impact is
washed out by the contraction), partition-split across the three DGE
queues (sync/scalar/gpsimd); outputs stream out per chunk and are
unscaled on host. The first L rows are
computed on host (segment 0 has no spin-up). All param-derived scalars
are per-partition [128,1] operands, so the compiled program is
input-independent.
"""

import numpy as np

T = 1048576
P = 128
NCORES = 8
L = 1024          # graded steps per lane
K = 192           # spin-up steps
W = K + L         # window length per lane (1216)
J0 = 64           # sweep-2 restart column
TC = T // NCORES  # steps per core
SLAB = TC + K     # u rows staged per core
NC_CONST = 17

# fixed model constants (match reference.py)
A, B, C_, D, E, F_, G, H = 0.5616, 0.3126, 48.43, 0.507, 55.0, 0.1538, 90.0, 0.16

# column chunking
CH_TILE = [(0, 192), (192, 832), (832, 1216)]               # DMA staging tiles
CH_DMA = [(0, 192), (192, 512), (512, 832), (832, 1216)]    # precompute grid
CH_A = [(0, 192), (192, 512), (512, 832), (832, 1215)]      # sweep-1 Y0 scan
CH_B = [(0, 512), (512, 1024), (1024, 1215)]                # sweep-1 Y1 scan
CH_VA = [(64, 512), (512, 1024), (1024, 1215)]              # v/a2 links
CH_V = [(64, 512), (512, 1024), (1024, 1215)]               # scanC/scanD
PC1 = [(0, 192), (192, 512), (512, 832), (832, 1024), (1024, 1215)]
PC2 = [(64, 512), (512, 1024), (1024, 1215)]
# X-column ranges streamed to output after sweep-2 scan chunks
OUT_CH = [(192, 513), (513, 1025), (1025, 1216)]

_cache = {}


def _build_nc():
    if "nc" in _cache:
        return _cache["nc"]
    from contextlib import ExitStack
    import concourse.bacc as bacc
    import concourse.tile as tile
    import concourse.mybir as mybir
    from bass_rust import AP

    f32 = mybir.dt.float32
    op = mybir.AluOpType
    ident = mybir.ActivationFunctionType.Identity
    nc = bacc.Bacc("TRN2", target_bir_lowering=False, debug=False,
                   enable_asserts=True, num_devices=NCORES)

    f16 = mybir.dt.float16
    uslab = nc.dram_tensor("uslab", [SLAB, 2], f16, kind="ExternalInput").ap()
    a1s = nc.dram_tensor("a1s", [SLAB, 1], f32, kind="ExternalInput").ap()
    cons = nc.dram_tensor("cons", [P, NC_CONST], f32, kind="ExternalInput").ap()
    iden = nc.dram_tensor("iden", [P, P], f32, kind="ExternalInput").ap()
    o0 = nc.dram_tensor("o0", [P, L], f32, kind="ExternalOutput").ap()
    o1 = nc.dram_tensor("o1", [P, L], f32, kind="ExternalOutput").ap()

    with tile.TileContext(nc) as tc, ExitStack() as ctx:
        pool = ctx.enter_context(tc.tile_pool(name="main", bufs=1))
        ppool = ctx.enter_context(tc.tile_pool(name="psum", bufs=1, space="PSUM"))
        t_u = [pool.tile([P, 2 * (hi - lo)], f16, name=f"u{d}", tag=f"u{d}")
               for d, (lo, hi) in enumerate(CH_TILE)]
        t_cons = pool.tile([P, NC_CONST], f32, name="cons", tag="cons")
        t_iden = pool.tile([P, P], f32, name="iden", tag="iden")
        t_scr = pool.tile([P, 8], f32, name="scr", tag="scr")

        def cst(i):
            return t_cons[:, i : i + 1]

        t_a1 = pool.tile([P, W], f32, name="a1", tag="a1")
        t_den = pool.tile([P, W], f32, name="den", tag="den")
        t_rec = pool.tile([P, W], f32, name="rec", tag="rec")
        t_SA = pool.tile([P, W], f32, name="SA", tag="SA")
        t_SC = pool.tile([P, W], f32, name="SC", tag="SC")
        t_SBp = pool.tile([P, W], f32, name="SBp", tag="SBp")
        t_b = pool.tile([P, W], f32, name="b", tag="b")
        t_v = pool.tile([P, W], f32, name="v", tag="v")
        t_a2 = pool.tile([P, W], f32, name="a2", tag="a2")
        t_Y0a = pool.tile([P, W], f32, name="Y0a", tag="Y0a")
        t_Y1a = pool.tile([P, W], f32, name="Y1a", tag="Y1a")
        t_Y0b = pool.tile([P, W], f32, name="Y0b", tag="Y0b")
        t_Y1b = pool.tile([P, W], f32, name="Y1b", tag="Y1b")
        t_c2s = pool.tile([P, W], f32, name="c2s", tag="c2s")
        t_c1p = ppool.tile([P, W - 1], f32, name="c1p", tag="c1p")
        t_c2p = ppool.tile([P, W - 1], f32, name="c2p", tag="c2p")

        def _utile(lo, hi):
            for d, (Lo, Hi) in enumerate(CH_TILE):
                if lo >= Lo and hi <= Hi:
                    return d, Lo
            raise AssertionError((lo, hi))

        def u0q(c):
            lo, hi = CH_DMA[c]
            d, Lo = _utile(lo, hi)
            return t_u[d][:, 2 * (lo - Lo) : 2 * (hi - Lo) : 2]

        def u1q(c):
            lo, hi = CH_DMA[c]
            d, Lo = _utile(lo, hi)
            return t_u[d][:, 2 * (lo - Lo) + 1 : 2 * (hi - Lo) : 2]

        # ---- preamble: DMA issue + engine warms --------------------------
        nc.gpsimd.memset(t_scr[:, 0:4], 0.0)
        nc.scalar.activation(t_scr[:, 0:1], t_scr[:, 1:2], ident,
                             bias=0.0, scale=1.0)
        # b tile (constant 1.0) built before the Pool-queue SWDGE work
        nc.gpsimd.memset(t_b[:], 1.0)
        nc.sync.dma_start(t_cons[:], cons[:])

        # input tiles (u fp16, a1 fp32) partition-split across the three
        # DGE queues; a1 feeds the sweep-1 scans directly (no ACT hop)
        def dma_half(d, half, eng):
            lo, hi = CH_TILE[d]
            w2 = 2 * (hi - lo)
            off = half * 64 * L * 2 + 2 * lo
            win = AP(uslab.tensor, off, [[L * 2, 64], [1, w2]])
            eng.dma_start(t_u[d][64 * half : 64 * (half + 1), :], win)

        def dma_a1_half(d, half, eng):
            lo, hi = CH_TILE[d]
            off = half * 64 * L + lo
            win = AP(a1s.tensor, off, [[L, 64], [1, hi - lo]])
            eng.dma_start(t_a1[64 * half : 64 * (half + 1), lo:hi], win)

        dma_a1_half(0, 0, nc.sync)
        dma_a1_half(0, 1, nc.scalar)
        dma_half(0, 0, nc.sync)
        dma_half(0, 1, nc.scalar)
        dma_half(1, 0, nc.gpsimd)
        dma_half(1, 1, nc.scalar)
        nc.scalar.dma_start(t_iden[:], iden[:])
        dma_half(2, 0, nc.sync)
        dma_half(2, 1, nc.gpsimd)
        dma_a1_half(2, 0, nc.sync)
        dma_a1_half(2, 1, nc.scalar)
        # Y0a column 0 = i0/c01 (read by the c1 PSUM accumulation)
        nc.scalar.activation(t_Y0a[:, 0:1], cst(15), ident, bias=0.0, scale=1.0)

        # ---- op builders -------------------------------------------------
        def act_pre(d):
            lo, hi = CH_DMA[d]
            if d in (1, 2):
                nc.scalar.activation(t_a1[:, lo:hi], u0q(d), ident,
                                     bias=cst(1), scale=cst(0))
            if d == 0:
                nc.vector.tensor_scalar(t_den[:, lo:hi], u1q(d), cst(2),
                                        cst(3), op.mult, op.add)
            else:
                nc.scalar.activation(t_den[:, lo:hi], u1q(d), ident,
                                     bias=cst(3), scale=cst(2))

        def act_post(d):
            lo, hi = CH_DMA[d]
            nc.scalar.activation(t_SA[:, lo:hi], t_rec[:, lo:hi], ident,
                                 bias=cst(5), scale=cst(4))
            nc.scalar.activation(t_SC[:, lo:hi], u0q(d), ident,
                                 bias=cst(7), scale=cst(6))
            nc.scalar.activation(t_SBp[:, lo:hi], u0q(d), ident,
                                 bias=cst(9), scale=cst(8))

        def rec(d):
            lo, hi = CH_DMA[d]
            nc.vector.reciprocal_approx_fast(t_rec[:, lo:hi], t_den[:, lo:hi])

        def mmSB(tp, lo, hi):
            # PSUM accumulation: tp[lo:hi] = SBpa + rec_n  (both +I matmuls)
            nc.tensor.matmul(tp[:, lo:hi], t_iden[:], t_SBp[:, lo:hi],
                             start=True, stop=False)
            nc.tensor.matmul(tp[:, lo:hi], t_iden[:], t_rec[:, lo:hi],
                             start=False, stop=False)

        def mmY(tp, src, lo, hi):
            # tp[lo:hi] += Y0 scan output (closes the accumulation group)
            nc.tensor.matmul(tp[:, lo:hi], t_iden[:], src[:, lo:hi],
                             start=False, stop=True)

        def v_(e):  # v = -c02*c01*Y0a + SC   (DVE stt)
            lo, hi = CH_VA[e]
            nc.vector.scalar_tensor_tensor(t_v[:, lo:hi], t_Y0a[:, lo:hi],
                                           cst(13), t_SC[:, lo:hi],
                                           op.mult, op.add)

        def a2_(e):  # a2 = -c03*al*Y1a + v   (DVE stt)
            lo, hi = CH_VA[e]
            nc.vector.scalar_tensor_tensor(t_a2[:, lo:hi], t_Y1a[:, lo:hi],
                                           cst(14), t_v[:, lo:hi],
                                           op.mult, op.add)

        def scanA(d):  # sweep-1 Y0
            lo, hi = CH_A[d]
            init = cst(15) if d == 0 else t_Y0a[:, lo : lo + 1]
            nc.vector.tensor_tensor_scan(t_Y0a[:, lo + 1 : hi + 1],
                                         t_a1[:, lo:hi], t_b[:, lo:hi],
                                         init, op.mult, op.add)

        def scanB(d):  # sweep-1 Y1 (data1 from PSUM)
            lo, hi = CH_B[d]
            init = cst(16) if d == 0 else t_Y1a[:, lo : lo + 1]
            nc.vector.tensor_tensor_scan(t_Y1a[:, lo + 1 : hi + 1],
                                         t_SA[:, lo:hi], t_c1p[:, lo:hi],
                                         init, op.mult, op.add)

        def scanC(e):  # sweep-2 Y0
            lo, hi = CH_V[e]
            init = t_Y0a[:, lo : lo + 1] if e == 0 else t_Y0b[:, lo : lo + 1]
            nc.vector.tensor_tensor_scan(t_Y0b[:, lo + 1 : hi + 1],
                                         t_a2[:, lo:hi], t_b[:, lo:hi],
                                         init, op.mult, op.add)

        def scanD(e):  # sweep-2 Y1 (data1 from PSUM; last chunk from SBUF)
            lo, hi = CH_V[e]
            init = t_Y1a[:, lo : lo + 1] if e == 0 else t_Y1b[:, lo : lo + 1]
            src_c2 = t_c2s if e == len(CH_V) - 1 else t_c2p
            nc.vector.tensor_tensor_scan(t_Y1b[:, lo + 1 : hi + 1],
                                         t_SA[:, lo:hi], src_c2[:, lo:hi],
                                         init, op.mult, op.add)

        def sbl():  # last-chunk SBa = SBpa + rec_n on DVE (off PE path)
            lo, hi = CH_V[-1]
            nc.vector.tensor_tensor(t_c2s[:, lo:hi], t_SBp[:, lo:hi],
                                    t_rec[:, lo:hi], op.add)

        def c2l():  # last-chunk c2 = SBa + Y0b on DVE
            lo, hi = CH_V[-1]
            nc.vector.tensor_tensor(t_c2s[:, lo:hi], t_c2s[:, lo:hi],
                                    t_Y0b[:, lo:hi], op.add)

        def out0(i):
            lo, hi = OUT_CH[i]
            nc.sync.dma_start(o0[:, lo - K : hi - K], t_Y0b[:, lo:hi])

        def out1(i):
            lo, hi = OUT_CH[i]
            if i == len(OUT_CH) - 1:
                # final chunk split across both queues to shorten the tail
                nc.scalar.dma_start(o1[0:64, lo - K : hi - K],
                                    t_Y1b[0:64, lo:hi])
                nc.sync.dma_start(o1[64:128, lo - K : hi - K],
                                  t_Y1b[64:128, lo:hi])
            else:
                nc.scalar.dma_start(o1[:, lo - K : hi - K], t_Y1b[:, lo:hi])

        def copy64():  # Y0b col 64 = Y0a col 64 (read by the c2 accumulation)
            nc.scalar.activation(t_Y0b[:, J0 : J0 + 1], t_Y0a[:, J0 : J0 + 1],
                                 ident, bias=0.0, scale=1.0)

        # ---- pipelined emission ------------------------------------------
        act_pre(0)
        rec(0)
        act_pre(1)
        act_post(0)
        mmSB(t_c1p, *PC1[0])
        scanA(0)
        rec(1)
        act_pre(2)
        act_post(1)
        mmY(t_c1p, t_Y0a, *PC1[0])
        mmSB(t_c1p, *PC1[1])
        scanA(1)
        rec(2)
        act_pre(3)
        act_post(2)
        copy64()
        mmY(t_c1p, t_Y0a, *PC1[1])
        mmSB(t_c1p, *PC1[2])
        scanB(0)
        scanA(2)
        mmY(t_c1p, t_Y0a, *PC1[2])
        rec(3)
        act_post(3)
        mmSB(t_c1p, *PC1[3])
        mmSB(t_c1p, *PC1[4])
        scanA(3)
        mmY(t_c1p, t_Y0a, *PC1[3])
        mmY(t_c1p, t_Y0a, *PC1[4])
        scanB(1)
        sbl()
        v_(0)
        a2_(0)
        mmSB(t_c2p, *PC2[0])
        scanC(0)
        mmY(t_c2p, t_Y0b, *PC2[0])
        scanB(2)
        v_(1)
        a2_(1)
        scanD(0)
        mmSB(t_c2p, *PC2[1])
        scanC(1)
        mmY(t_c2p, t_Y0b, *PC2[1])
        out0(0)
        v_(2)
        a2_(2)
        scanD(1)
        out1(0)
        scanC(2)
        c2l()
        out0(1)
        scanD(2)
        out1(1)
        out0(2)
        out1(2)

    nc.compile()
    _cache["nc"] = nc
    return nc


def _derive(params, x0):
    M, Cc, UA2, Cp, lam, lams, F1, X1p, F3, T1, T200 = [float(params[i]) for i in range(11)]
    UA1 = H * (F1 + F3)
    k1 = (UA1 + F1 * Cp) / lam
    p_ = k1 * B
    q_ = k1 * A
    alpha_u = UA1 * F_ / lam
    alpha_c = (UA1 * G + F1 * Cp * T1) / lam - k1 * C_
    c01 = F1 * X1p / M
    c02 = p_ / M
    c03 = q_ / M
    a10 = -p_ / Cc
    cA2 = -D / (lam * Cc)
    cA1 = 1.0 - q_ / Cc
    cB2 = alpha_u / Cc
    cB1 = alpha_c / Cc
    cB3 = -(E - T200) / (lam * Cc)
    cC2 = alpha_u / M
    cC1 = 1.0 - (F1 - alpha_c) / M
    i0, i1 = float(x0[0]), float(x0[1])
    al = a10 * c01                 # alpha (< 0)
    s_ = -cB3 * UA2 * UA2          # > 0

    cv = np.zeros(NC_CONST, np.float64)
    cv[0] = cC2                           # a1 scale
    cv[1] = cC1 - (c02 * i0 + c03 * i1)   # a1 bias
    cv[2] = 2.0 * Cp * al / s_            # den_n scale (negative)
    cv[3] = UA2 * al / s_                 # den_n bias (negative)
    cv[4] = -cA2 * UA2 * UA2 * al / s_    # SA scale (of rec_n)
    cv[5] = cA1 + cA2 * UA2               # SA bias
    cv[6] = cC2                           # SC scale
    cv[7] = cC1                           # SC bias
    cv[8] = cB2 / al                      # SBpa scale
    cv[9] = (cB1 + cB3 * UA2) / al        # SBpa bias
    cv[13] = -c02 * c01                   # v scalar
    cv[14] = -c03 * al                    # a2 scalar
    cv[15] = i0 / c01
    cv[16] = i1 / al
    return cv.astype(np.float32), np.float32(c01), np.float32(al)


def _make_in_maps(u, cons):
    u = np.ascontiguousarray(u, np.float32)
    consv = cons
    cons = np.tile(cons[None, :], (P, 1))
    eye = np.eye(P, dtype=np.float32)
    in_maps = []
    for c in range(NCORES):
        if c == 0:
            slab = np.concatenate([np.repeat(u[0:1], K, axis=0), u[0:TC]], axis=0)
        else:
            slab = u[c * TC - K : c * TC + TC]
        a1slab = (slab[:, 0] * consv[0] + consv[1]).astype(np.float32)
        in_maps.append({
            "uslab": np.ascontiguousarray(slab, np.float16),
            "a1s": np.ascontiguousarray(a1slab[:, None]),
            "cons": cons,
            "iden": eye,
        })
    return in_maps


def _host_head(u, x0, params, n):
    # exact fp32 simulation of the first n steps (segment 0 has no spin-up)
    f = np.float32
    M, Cc, UA2, Cp, lam, lams, F1, X1p, F3, T1, T200 = [f(params[i]) for i in range(11)]
    out = np.empty((n, 2), f)
    s0, s1 = f(x0[0]), f(x0[1])
    fA, fB, fC, fD, fE, fF, fG, fH = f(A), f(B), f(C_), f(D), f(E), f(F_), f(G), f(H)
    one, two = f(1.0), f(2.0)
    UA1 = fH * (F1 + F3)
    for t in range(n):
        out[t, 0] = s0
        out[t, 1] = s1
        u0, u1 = f(u[t, 0]), f(u[t, 1])
        T2 = fA * s1 + fB * s0 + fC
        T3 = fD * s1 + fE
        T100 = fF * u0 + fG
        Q100 = UA1 * (T100 - T2)
        Q200 = UA2 * (T3 - T200) / (one + UA2 / (two * Cp * u1))
        F5 = Q200 / lam
        F4 = (Q100 - F1 * Cp * (T2 - T1)) / lam
        F2 = F1 - F4
        X2d = (F1 * X1p - F2 * s0) / M
        P2d = (F4 - F5) / Cc
        s0 = s0 + X2d
        s1 = s1 + P2d
    return out


def _assemble(results, head, c01, al):
    out = np.empty((T, 2), np.float32)
    for c in range(NCORES):
        out[c * TC : (c + 1) * TC, 0] = results[c]["o0"].reshape(-1) * c01
        out[c * TC : (c + 1) * TC, 1] = results[c]["o1"].reshape(-1) * al
    out[0:L] = head
    return out


def run(u_forced, x0, params, trace=False):
    from concourse.bass_utils import run_bass_kernel_spmd
    nc = _build_nc()
    cons, c01, al = _derive(params, x0)
    in_maps = _make_in_maps(u_forced, cons)
    head = _host_head(u_forced, x0, params, L)
    res = run_bass_kernel_spmd(nc, in_maps, list(range(NCORES)), trace=trace)
    return _assemble(res.results, head, c01, al), res


def kernel(u_forced, x0, params):
    out, _ = run(u_forced, x0, params, trace=False)
    return out



# revision 7
# speedup vs baseline: 1.0232x; 1.0232x over previous
"""Trainium2 Bass kernel for the CSTR (evaporator) 1M-step scan.

Parallel-in-time, two-level resolution. The per-step map is contractive
(slow mode ~0.9665/step), so the trajectory splits into 1024 windows
(8 cores x 128 lanes) of L=1024 graded steps plus K=160 spin-up steps
(W=1184). Per lane:

  sweep 1 (linearization source) runs at QUARTER resolution: the a1/SA
  coefficients are composed over 4 consecutive steps on the host
  (elementwise, like the baseline's a1s precompute) and shipped as a
  coarse package (A4,B4,SA4,gsp,Qc); the device runs two 296-col scans
  (Y0c, Y1c) and forms w_c = cv13*Y0c + cv14*Y1c.

  sweep 2 (graded) is STEP-DOUBLED: even-grid scans of 592 cols with
  exact elementwise odd recovery. a2_{e,o} = w_c (broadcast) + SC_{e,o};
  Y0b_e = scan(a2_e*a2_o, a2_o+1); odd Y0b = a2_e*Y0b_e (+1 on host).
  c2_e = Y0b_e + SBpa_e - rec'_e accumulates in PSUM via fp32r identity
  matmuls (1 cycle/row); Bd2c = SA_o*c2_e + Y0b_o + SBpa_o(+1) - rec'_o
  likewise; Y1b_e = scan(SA_e*SA_o, Bd2c); odd Y1b = SA_e*Y1b_e + c2_e.

Inputs ship as de-interleaved fp16 planes (u0e/u0o/u1e/u1o); outputs
stream out as even/odd planes and are interleaved/scaled on the host.
The first L rows are computed on the host (window 0 has no spin-up).
All param-derived scalars are per-partition [128,1] operands, so the
compiled program is input-independent.
"""

import numpy as np

T = 1048576
P = 128
NCORES = 8
L = 1024          # graded steps per lane
K = 160           # spin-up steps
W = K + L         # window length per lane (1184)
W2 = W // 2       # half grid (592)
WC = W // 4       # coarse grid (296)
GO = K // 2       # graded offset on half grid (80)
LH = L // 2       # graded half length (512)
TC = T // NCORES  # steps per core
SLAB2 = TC // 2 + K // 2
SLAB4 = TC // 4 + K // 4
NC_CONST = 13

# fixed model constants (match reference.py)
A, B, C_, D, E, F_, G, H = 0.5616, 0.3126, 48.43, 0.507, 55.0, 0.1538, 90.0, 0.16

# chunking of the half grid
CH = [(0, 296), (296, 592)]

_cache = {}


def _build_nc():
    if "nc" in _cache:
        return _cache["nc"]
    from contextlib import ExitStack
    import concourse.bacc as bacc
    import concourse.tile as tile
    import concourse.mybir as mybir
    from bass_rust import AP

    f32 = mybir.dt.float32
    f32r = mybir.dt.float32r
    f16 = mybir.dt.float16
    op = mybir.AluOpType
    ident = mybir.ActivationFunctionType.Identity
    nc = bacc.Bacc("TRN2", target_bir_lowering=False, debug=False,
                   enable_asserts=True, num_devices=NCORES)

    # DRAM I/O
    d_u0e = nc.dram_tensor("u0e", [SLAB2, 1], f16, kind="ExternalInput").ap()
    d_u0o = nc.dram_tensor("u0o", [SLAB2, 1], f16, kind="ExternalInput").ap()
    d_u1e = nc.dram_tensor("u1e", [SLAB2, 1], f16, kind="ExternalInput").ap()
    d_u1o = nc.dram_tensor("u1o", [SLAB2, 1], f16, kind="ExternalInput").ap()
    d_A4 = nc.dram_tensor("A4", [SLAB4, 1], f32, kind="ExternalInput").ap()
    d_B4 = nc.dram_tensor("B4", [SLAB4, 1], f32, kind="ExternalInput").ap()
    d_SA4 = nc.dram_tensor("SA4", [SLAB4, 1], f32, kind="ExternalInput").ap()
    d_gsp = nc.dram_tensor("gsp", [SLAB4, 1], f32, kind="ExternalInput").ap()
    d_Qc = nc.dram_tensor("Qc", [SLAB4, 1], f32, kind="ExternalInput").ap()
    cons = nc.dram_tensor("cons", [P, NC_CONST], f32, kind="ExternalInput").ap()
    iden = nc.dram_tensor("iden", [P, P], f32, kind="ExternalInput").ap()
    nide = nc.dram_tensor("nide", [P, P], f32, kind="ExternalInput").ap()
    o0e = nc.dram_tensor("o0e", [P, LH], f32, kind="ExternalOutput").ap()
    o0o = nc.dram_tensor("o0o", [P, LH], f32, kind="ExternalOutput").ap()
    o1e = nc.dram_tensor("o1e", [P, LH], f32, kind="ExternalOutput").ap()
    o1o = nc.dram_tensor("o1o", [P, LH], f32, kind="ExternalOutput").ap()

    with tile.TileContext(nc) as tc, ExitStack() as ctx:
        pool = ctx.enter_context(tc.tile_pool(name="main", bufs=1))
        ppool = ctx.enter_context(tc.tile_pool(name="psum", bufs=1, space="PSUM"))

        t_u0e = pool.tile([P, W2], f16, name="u0e", tag="u0e")
        t_u0o = pool.tile([P, W2], f16, name="u0o", tag="u0o")
        t_u1e = pool.tile([P, W2], f16, name="u1e", tag="u1e")
        t_u1o = pool.tile([P, W2], f16, name="u1o", tag="u1o")
        t_A4 = pool.tile([P, WC], f32, name="A4", tag="A4")
        t_B4 = pool.tile([P, WC], f32, name="B4", tag="B4")
        t_SA4 = pool.tile([P, WC], f32, name="SA4", tag="SA4")
        t_gsp = pool.tile([P, WC], f32, name="gsp", tag="gsp")
        t_Qc = pool.tile([P, WC], f32, name="Qc", tag="Qc")
        t_cons = pool.tile([P, NC_CONST], f32, name="cons", tag="cons")
        t_iden = pool.tile([P, P], f32, name="iden", tag="iden")
        t_nide = pool.tile([P, P], f32, name="nide", tag="nide")
        t_scr = pool.tile([P, 8], f32, name="scr", tag="scr")

        t_dene = pool.tile([P, W2], f32, name="dene", tag="dene")
        t_deno = pool.tile([P, W2], f32, name="deno", tag="deno")
        t_rece = pool.tile([P, W2], f32, name="rece", tag="rece")
        t_reco = pool.tile([P, W2], f32, name="reco", tag="reco")
        t_SAe = pool.tile([P, W2], f32, name="SAe", tag="SAe")
        t_SAo = pool.tile([P, W2], f32, name="SAo", tag="SAo")
        t_SBpe = pool.tile([P, W2], f32, name="SBpe", tag="SBpe")
        t_SBpo = pool.tile([P, W2], f32, name="SBpo", tag="SBpo")
        t_SCe = pool.tile([P, W2], f32, name="SCe", tag="SCe")
        t_SCo = pool.tile([P, W2], f32, name="SCo", tag="SCo")
        t_SA2 = pool.tile([P, W2], f32, name="SA2", tag="SA2")

        t_Y0c = pool.tile([P, WC], f32, name="Y0c", tag="Y0c")
        t_c1c = pool.tile([P, WC], f32, name="c1c", tag="c1c")
        t_Y1c = pool.tile([P, WC], f32, name="Y1c", tag="Y1c")
        t_wc = pool.tile([P, WC], f32, name="wc", tag="wc")

        t_a2e = pool.tile([P, W2], f32, name="a2e", tag="a2e")
        t_a2o = pool.tile([P, W2], f32, name="a2o", tag="a2o")
        t_Ad2 = pool.tile([P, W2], f32, name="Ad2", tag="Ad2")
        t_Bd2 = pool.tile([P, W2], f32, name="Bd2", tag="Bd2")
        t_M1 = pool.tile([P, W2], f32, name="M1", tag="M1")
        t_M2 = pool.tile([P, W2], f32, name="M2", tag="M2")
        t_Y0be = pool.tile([P, W2], f32, name="Y0be", tag="Y0be")
        t_Y0bo = pool.tile([P, W2], f32, name="Y0bo", tag="Y0bo")
        t_Y1be = pool.tile([P, W2], f32, name="Y1be", tag="Y1be")
        t_Y1bo = pool.tile([P, W2], f32, name="Y1bo", tag="Y1bo")

        # PSUM: one tile per chunk, each within a single 2KB bank
        p_c2e = [ppool.tile([P, hi - lo], f32, name=f"c2e{d}", tag=f"c2e{d}")
                 for d, (lo, hi) in enumerate(CH)]
        p_bd = [ppool.tile([P, hi - lo], f32, name=f"bd{d}", tag=f"bd{d}")
                for d, (lo, hi) in enumerate(CH)]

        def cst(i):
            return t_cons[:, i : i + 1]

        # ---- preamble: engine warms + DMA issue --------------------------
        nc.gpsimd.memset(t_scr[:, 0:4], 0.0)
        nc.scalar.activation(t_scr[:, 0:1], t_scr[:, 1:2], ident,
                             bias=0.0, scale=1.0)
        nc.scalar.dma_start(t_cons[:], cons[:])

        def dma_plane(eng, dst, src, stride, n, half):
            off = half * 64 * stride
            win = AP(src.tensor, off, [[stride, 64], [1, n]])
            eng.dma_start(dst[64 * half : 64 * (half + 1), :], win)

        # coarse package first (feeds the DVE scan chain)
        dma_plane(nc.sync, t_A4, d_A4, L // 4, WC, 0)
        dma_plane(nc.sync, t_A4, d_A4, L // 4, WC, 1)
        dma_plane(nc.scalar, t_B4, d_B4, L // 4, WC, 0)
        dma_plane(nc.scalar, t_B4, d_B4, L // 4, WC, 1)
        dma_plane(nc.gpsimd, t_SA4, d_SA4, L // 4, WC, 0)
        dma_plane(nc.gpsimd, t_SA4, d_SA4, L // 4, WC, 1)
        dma_plane(nc.sync, t_u1e, d_u1e, L // 2, W2, 0)
        dma_plane(nc.sync, t_u1e, d_u1e, L // 2, W2, 1)
        dma_plane(nc.scalar, t_u1o, d_u1o, L // 2, W2, 0)
        dma_plane(nc.scalar, t_u1o, d_u1o, L // 2, W2, 1)
        dma_plane(nc.gpsimd, t_gsp, d_gsp, L // 4, WC, 0)
        dma_plane(nc.gpsimd, t_gsp, d_gsp, L // 4, WC, 1)
        dma_plane(nc.gpsimd, t_Qc, d_Qc, L // 4, WC, 0)
        dma_plane(nc.gpsimd, t_Qc, d_Qc, L // 4, WC, 1)
        dma_plane(nc.sync, t_u0e, d_u0e, L // 2, W2, 0)
        dma_plane(nc.sync, t_u0e, d_u0e, L // 2, W2, 1)
        dma_plane(nc.scalar, t_u0o, d_u0o, L // 2, W2, 0)
        dma_plane(nc.scalar, t_u0o, d_u0o, L // 2, W2, 1)
        nc.gpsimd.dma_start(t_iden[:], iden[:])
        nc.gpsimd.dma_start(t_nide[:], nide[:])

        # scan column-0 inits
        nc.scalar.activation(t_Y0c[:, 0:1], cst(10), ident, bias=0.0, scale=1.0)
        nc.scalar.activation(t_Y1c[:, 0:1], cst(12), ident, bias=0.0, scale=1.0)
        nc.scalar.activation(t_Y0be[:, 0:1], cst(10), ident, bias=0.0, scale=1.0)
        nc.scalar.activation(t_Y1be[:, 0:1], cst(11), ident, bias=0.0, scale=1.0)

        # ---- op builders -------------------------------------------------
        def den_(d, which):
            lo, hi = CH[d]
            t_u, t_den = (t_u1e, t_dene) if which == "e" else (t_u1o, t_deno)
            nc.scalar.activation(t_den[:, lo:hi], t_u[:, lo:hi], ident,
                                 bias=cst(1), scale=cst(0))

        def rec_(d, which):
            lo, hi = CH[d]
            t_den, t_rec = (t_dene, t_rece) if which == "e" else (t_deno, t_reco)
            nc.vector.reciprocal_approx_fast(t_rec[:, lo:hi], t_den[:, lo:hi])

        def SA_(d, which):
            lo, hi = CH[d]
            t_rec, t_SA = (t_rece, t_SAe) if which == "e" else (t_reco, t_SAo)
            nc.scalar.activation(t_SA[:, lo:hi], t_rec[:, lo:hi], ident,
                                 bias=cst(3), scale=cst(2))

        def SBp_(d, which):
            lo, hi = CH[d]
            if which == "e":
                nc.scalar.activation(t_SBpe[:, lo:hi], t_u0e[:, lo:hi], ident,
                                     bias=cst(5), scale=cst(4))
            else:
                nc.scalar.activation(t_SBpo[:, lo:hi], t_u0o[:, lo:hi], ident,
                                     bias=cst(6), scale=cst(4))

        def SC_(d, which):
            lo, hi = CH[d]
            t_u, t_SC = (t_u0e, t_SCe) if which == "e" else (t_u0o, t_SCo)
            nc.scalar.activation(t_SC[:, lo:hi], t_u[:, lo:hi], ident,
                                 bias=cst(8), scale=cst(7))

        def scanY0c():
            nc.vector.tensor_tensor_scan(t_Y0c[:, 1:WC], t_A4[:, 0:WC-1],
                                         t_B4[:, 0:WC-1], cst(10),
                                         op.mult, op.add)

        def c1c_():
            nc.vector.tensor_tensor(t_c1c[:], t_gsp[:], t_Y0c[:], op.mult)
            nc.vector.tensor_tensor(t_c1c[:], t_c1c[:], t_Qc[:], op.add)

        def scanY1c():
            nc.vector.tensor_tensor_scan(t_Y1c[:, 1:WC], t_SA4[:, 0:WC-1],
                                         t_c1c[:, 0:WC-1], cst(12),
                                         op.mult, op.add)

        def wc_():
            nc.vector.scalar_tensor_tensor(t_wc[:], t_Y0c[:], cst(9),
                                           t_Y1c[:], op.mult, op.add)

        def wc_view(d):
            # broadcast each w_c col to 2 half-grid cols (stride-0 inner dim)
            lo, hi = CH[d]
            n = (hi - lo) // 2
            return t_wc[:, lo // 2 : lo // 2 + n].unsqueeze(2).broadcast_to([P, n, 2])

        def a2_(d, which):
            lo, hi = CH[d]
            t_SC, t_a2 = (t_SCe, t_a2e) if which == "e" else (t_SCo, t_a2o)
            nc.vector.tensor_tensor(t_a2[:, lo:hi], wc_view(d),
                                    t_SC[:, lo:hi], op.add)

        def Ad2_(d):
            lo, hi = CH[d]
            nc.vector.tensor_tensor(t_Ad2[:, lo:hi], t_a2e[:, lo:hi],
                                    t_a2o[:, lo:hi], op.mult)

        def Bd2_(d):
            lo, hi = CH[d]
            nc.scalar.activation(t_Bd2[:, lo:hi], t_a2o[:, lo:hi], ident,
                                 bias=1.0, scale=1.0)

        def SA2_(d):
            lo, hi = CH[d]
            nc.vector.tensor_tensor(t_SA2[:, lo:hi], t_SAe[:, lo:hi],
                                    t_SAo[:, lo:hi], op.mult)

        def scanY0b(d):
            lo, hi = CH[d]
            init = cst(10) if d == 0 else t_Y0be[:, lo : lo + 1]
            n = hi - 1 if d == len(CH) - 1 else hi
            nc.vector.tensor_tensor_scan(t_Y0be[:, lo + 1 : n + 1],
                                         t_Ad2[:, lo:n], t_Bd2[:, lo:n],
                                         init, op.mult, op.add)

        def Y0bo_(d):
            lo, hi = CH[d]
            nc.gpsimd.tensor_tensor(t_Y0bo[:, lo:hi], t_a2e[:, lo:hi],
                                    t_Y0be[:, lo:hi], op.mult)

        def c2e_(d):
            lo, hi = CH[d]
            tp = p_c2e[d][:]
            nc.tensor.matmul(tp, t_iden[:],
                             t_Y0be[:, lo:hi],
                             start=True, stop=False)
            nc.tensor.matmul(tp, t_iden[:],
                             t_SBpe[:, lo:hi],
                             start=False, stop=False)
            nc.tensor.matmul(tp, t_nide[:],
                             t_rece[:, lo:hi],
                             start=False, stop=True)

        def M1_(d):
            lo, hi = CH[d]
            nc.vector.tensor_tensor(t_M1[:, lo:hi], t_SAo[:, lo:hi],
                                    p_c2e[d][:], op.mult)

        def bd_(d):
            lo, hi = CH[d]
            tp = p_bd[d][:]
            nc.tensor.matmul(tp, t_iden[:],
                             t_M1[:, lo:hi],
                             start=True, stop=False)
            nc.tensor.matmul(tp, t_iden[:],
                             t_Y0bo[:, lo:hi],
                             start=False, stop=False)
            nc.tensor.matmul(tp, t_iden[:],
                             t_SBpo[:, lo:hi],
                             start=False, stop=False)
            nc.tensor.matmul(tp, t_nide[:],
                             t_reco[:, lo:hi],
                             start=False, stop=True)

        def scanY1b(d):
            lo, hi = CH[d]
            init = cst(11) if d == 0 else t_Y1be[:, lo : lo + 1]
            n = hi - 1 if d == len(CH) - 1 else hi
            nc.vector.tensor_tensor_scan(t_Y1be[:, lo + 1 : n + 1],
                                         t_SA2[:, lo:n], p_bd[d][:, 0 : n - lo],
                                         init, op.mult, op.add)

        def M2_(d):
            lo, hi = CH[d]
            nc.vector.tensor_tensor(t_M2[:, lo:hi], t_SAe[:, lo:hi],
                                    t_Y1be[:, lo:hi], op.mult)

        def Y1bo_(d):
            lo, hi = CH[d]
            nc.vector.tensor_tensor(t_Y1bo[:, lo:hi], t_M2[:, lo:hi],
                                    p_c2e[d][:], op.add)

        def outs(d):
            lo, hi = CH[d]
            olo, ohi = max(lo, GO) - GO, hi - GO
            slo = olo + GO
            nc.sync.dma_start(o0e[:, olo:ohi], t_Y0be[:, slo:hi])
            nc.scalar.dma_start(o0o[:, olo:ohi], t_Y0bo[:, slo:hi])
            nc.sync.dma_start(o1e[:, olo:ohi], t_Y1be[:, slo:hi])
            nc.scalar.dma_start(o1o[:, olo:ohi], t_Y1bo[:, slo:hi])

        # ---- pipelined emission ------------------------------------------
        den_(0, "e"); den_(0, "o")
        scanY0c()
        den_(1, "e"); den_(1, "o")
        rec_(0, "e"); rec_(0, "o")
        c1c_()
        SA_(0, "e"); SA_(0, "o")
        scanY1c()
        SBp_(0, "e"); SBp_(0, "o")
        rec_(1, "e"); rec_(1, "o")
        wc_()
        SC_(0, "e"); SC_(0, "o")
        SA_(1, "e"); SA_(1, "o")
        a2_(0, "e"); a2_(0, "o")
        SBp_(1, "e"); SBp_(1, "o")
        Ad2_(0)
        Bd2_(0)
        SA2_(0)
        scanY0b(0)
        SC_(1, "e"); SC_(1, "o")
        a2_(1, "e"); a2_(1, "o")
        c2e_(0)
        Y0bo_(0)
        Ad2_(1)
        Bd2_(1)
        M1_(0)
        SA2_(1)
        bd_(0)
        scanY0b(1)
        c2e_(1)
        Y0bo_(1)
        scanY1b(0)
        M1_(1)
        M2_(0)
        bd_(1)
        Y1bo_(0)
        outs(0)
        scanY1b(1)
        M2_(1)
        Y1bo_(1)
        outs(1)

    nc.compile()
    _cache["nc"] = nc
    return nc


def _derive(params, x0):
    M, Cc, UA2, Cp, lam, lams, F1, X1p, F3, T1, T200 = [float(params[i]) for i in range(11)]
    UA1 = H * (F1 + F3)
    k1 = (UA1 + F1 * Cp) / lam
    p_ = k1 * B
    q_ = k1 * A
    alpha_u = UA1 * F_ / lam
    alpha_c = (UA1 * G + F1 * Cp * T1) / lam - k1 * C_
    c01 = F1 * X1p / M
    c02 = p_ / M
    c03 = q_ / M
    a10 = -p_ / Cc
    cA2 = -D / (lam * Cc)
    cA1 = 1.0 - q_ / Cc
    cB2 = alpha_u / Cc
    cB1 = alpha_c / Cc
    cB3 = -(E - T200) / (lam * Cc)
    cC2 = alpha_u / M
    cC1 = 1.0 - (F1 - alpha_c) / M
    i0, i1 = float(x0[0]), float(x0[1])
    al = a10 * c01                 # alpha (< 0)
    s_ = -cB3 * UA2 * UA2          # > 0

    cv = np.zeros(17, np.float64)
    cv[0] = cC2                           # a1 scale
    cv[1] = cC1 - (c02 * i0 + c03 * i1)   # a1 bias
    cv[2] = 2.0 * Cp * al / s_            # den scale (negative)
    cv[3] = UA2 * al / s_                 # den bias (negative)
    cv[4] = -cA2 * UA2 * UA2 * al / s_    # SA scale (of rec)
    cv[5] = cA1 + cA2 * UA2               # SA bias
    cv[6] = cC2                           # SC scale
    cv[7] = cC1                           # SC bias
    cv[8] = cB2 / al                      # SBpa scale
    cv[9] = (cB1 + cB3 * UA2) / al        # SBpa bias
    cv[13] = -c02 * c01                   # w scalar (Y0 coeff)
    cv[14] = -c03 * al                    # Y1 coeff (folded into gs)
    cv[15] = i0 / c01
    cv[16] = i1 / al
    return cv, np.float32(c01), np.float32(al)


def _device_cons(cv):
    c = np.zeros(NC_CONST, np.float64)
    c[0] = -cv[2]           # den scale (positive-den variant)
    c[1] = -cv[3]           # den bias
    c[2] = -cv[4]           # SA scale (of positive rec')
    c[3] = cv[5]            # SA bias
    c[4] = cv[8]            # SBpa scale
    c[5] = cv[9]            # SBpa_e bias
    c[6] = cv[9] + 1.0      # SBpa_o bias (+1 fold)
    c[7] = cv[6]            # SC scale
    c[8] = cv[7]            # SC bias
    c[9] = cv[13]           # w_c scalar
    c[10] = cv[15]          # Y0 init
    c[11] = cv[16]          # Y1b init
    c[12] = cv[16] * cv[14] # Y1c init (scaled)
    return c.astype(np.float32)


def _make_in_maps(u, cv):
    f = np.float32
    uq = np.ascontiguousarray(u, f).astype(np.float16)
    # padded (K leading repeat rows) fp32 view for package composition
    up = np.concatenate([np.repeat(uq[0:1], K, axis=0), uq], axis=0).astype(f)
    TP = T + K

    a1 = (f(cv[0]) * up[:, 0] + f(cv[1])).astype(f)
    den = (f(cv[2]) * up[:, 1] + f(cv[3])).astype(f)
    rec = (1.0 / den).astype(f)
    SA = (f(cv[4]) * rec + f(cv[5])).astype(f)
    SBr = (f(cv[8]) * up[:, 0] + f(cv[9]) + rec).astype(f)

    # coarse composition (b=1 for the a1 scan)
    A2 = (a1[0::2] * a1[1::2]).astype(f)
    B2 = (a1[1::2] + 1.0).astype(f)
    A4 = (A2[0::2] * A2[1::2]).astype(f)
    B4 = (A2[1::2] * B2[0::2] + B2[1::2]).astype(f)
    SA4 = (SA[0::4] * SA[1::4] * SA[2::4] * SA[3::4]).astype(f)
    SAc = SA[0::4]
    gs = (1.0 + SAc * (1.0 + SAc * (1.0 + SAc))).astype(f)
    gsp = (f(cv[14]) * gs).astype(f)
    Qc = (gsp * SBr[0::4]).astype(f)

    u0e = up[0::2, 0].astype(np.float16)
    u0o = up[1::2, 0].astype(np.float16)
    u1e = up[0::2, 1].astype(np.float16)
    u1o = up[1::2, 1].astype(np.float16)

    cons = np.tile(_device_cons(cv)[None, :], (P, 1))
    eye = np.eye(P, dtype=f)
    neye = -eye

    in_maps = []
    for c in range(NCORES):
        r2 = c * TC // 2
        r4 = c * TC // 4
        in_maps.append({
            "u0e": np.ascontiguousarray(u0e[r2 : r2 + SLAB2])[:, None],
            "u0o": np.ascontiguousarray(u0o[r2 : r2 + SLAB2])[:, None],
            "u1e": np.ascontiguousarray(u1e[r2 : r2 + SLAB2])[:, None],
            "u1o": np.ascontiguousarray(u1o[r2 : r2 + SLAB2])[:, None],
            "A4": np.ascontiguousarray(A4[r4 : r4 + SLAB4])[:, None],
            "B4": np.ascontiguousarray(B4[r4 : r4 + SLAB4])[:, None],
            "SA4": np.ascontiguousarray(SA4[r4 : r4 + SLAB4])[:, None],
            "gsp": np.ascontiguousarray(gsp[r4 : r4 + SLAB4])[:, None],
            "Qc": np.ascontiguousarray(Qc[r4 : r4 + SLAB4])[:, None],
            "cons": cons,
            "iden": eye,
            "nide": neye,
        })
    return in_maps


def _host_head(u, x0, params, n):
    # exact fp32 simulation of the first n steps (window 0 has no spin-up)
    f = np.float32
    M, Cc, UA2, Cp, lam, lams, F1, X1p, F3, T1, T200 = [f(params[i]) for i in range(11)]
    out = np.empty((n, 2), f)
    s0, s1 = f(x0[0]), f(x0[1])
    fA, fB, fC, fD, fE, fF, fG, fH = f(A), f(B), f(C_), f(D), f(E), f(F_), f(G), f(H)
    one, two = f(1.0), f(2.0)
    UA1 = fH * (F1 + F3)
    for t in range(n):
        out[t, 0] = s0
        out[t, 1] = s1
        u0, u1 = f(u[t, 0]), f(u[t, 1])
        T2 = fA * s1 + fB * s0 + fC
        T3 = fD * s1 + fE
        T100 = fF * u0 + fG
        Q100 = UA1 * (T100 - T2)
        Q200 = UA2 * (T3 - T200) / (one + UA2 / (two * Cp * u1))
        F5 = Q200 / lam
        F4 = (Q100 - F1 * Cp * (T2 - T1)) / lam
        F2 = F1 - F4
        X2d = (F1 * X1p - F2 * s0) / M
        P2d = (F4 - F5) / Cc
        s0 = s0 + X2d
        s1 = s1 + P2d
    return out


def _assemble(results, head, c01, al):
    out = np.empty((T, 2), np.float32)
    e0 = np.stack([r["o0e"] for r in results])  # [NCORES, P, LH]
    o0 = np.stack([r["o0o"] for r in results])
    e1 = np.stack([r["o1e"] for r in results])
    o1 = np.stack([r["o1o"] for r in results])
    x0v = np.empty((T, 2), np.float32)
    x0v[0::2, 0] = e0.reshape(-1)
    x0v[1::2, 0] = o0.reshape(-1) + 1.0
    x0v[0::2, 1] = e1.reshape(-1)
    x0v[1::2, 1] = o1.reshape(-1)
    out[:, 0] = x0v[:, 0] * c01
    out[:, 1] = x0v[:, 1] * al
    out[0:L] = head
    return out


def run(u_forced, x0, params, trace=False):
    from concourse.bass_utils import run_bass_kernel_spmd
    nc = _build_nc()
    cv, c01, al = _derive(params, x0)
    in_maps = _make_in_maps(u_forced, cv)
    head = _host_head(u_forced, x0, params, L)
    res = run_bass_kernel_spmd(nc, in_maps, list(range(NCORES)), trace=trace)
    return _assemble(res.results, head, c01, al), res


def kernel(u_forced, x0, params):
    out, _ = run(u_forced, x0, params, trace=False)
    return out


# revision 11
# speedup vs baseline: 1.1627x; 1.1363x over previous
"""Trainium2 Bass kernel for the CSTR (evaporator) 1M-step scan.

Parallel-in-time, two-level resolution. The per-step map is contractive
(slow mode ~0.9665/step), so the trajectory splits into 1024 windows
(8 cores x 128 lanes) of L=1024 graded steps plus K=160 spin-up steps
(W=1184). Per lane:

  sweep 1 (linearization source) runs at QUARTER resolution: the a1/SA
  coefficients are composed over 4 consecutive steps on the host
  (elementwise, like the baseline's a1s precompute) and shipped as a
  coarse package (A4,B4,SA4,gsp,Qc); the device runs two 296-col scans
  (Y0c, Y1c) and forms w_c = cv13*Y0c + Y1c (cv14 folded into gsp/Qc).

  sweep 2 (graded) is STEP-DOUBLED: even-grid scans of 592 cols with
  exact elementwise odd recovery. a2_{e,o} = w_c (broadcast) + SC_{e,o};
  Y0b_e = scan(a2_e*a2_o, a2_o+1); odd Y0b = a2_e*Y0b_e (+1 on host).
  c2_e = Y0b_e + SBpa_e - rec'_e accumulates in PSUM via fp16 identity
  matmuls (1 cycle/row; fp16 operand noise ~5e-4 is washed out);
  Bd2c = SA_o*c2_e + Y0b_o + SBpa_o(+1) - rec'_o likewise;
  Y1b_e = scan(SA_e*SA_o, Bd2c); odd Y1b = SA_e*Y1b_e + c2_e.

Inputs ship as de-interleaved fp16 planes (u0e/u0o/u1e/u1o); scan/odd
outputs are written fp16 and stream out as even/odd planes, interleaved
and scaled on the host. The first L rows are computed on the host
(window 0 has no spin-up). All param-derived scalars are per-partition
[128,1] operands, so the compiled program is input-independent.
"""

import numpy as np

T = 1048576
P = 128
NCORES = 8
L = 1024          # graded steps per lane
K = 160           # spin-up steps
W = K + L         # window length per lane (1184)
W2 = W // 2       # half grid (592)
WC = W // 4       # coarse grid (296)
GO = K // 2       # graded offset on half grid (80)
LH = L // 2       # graded half length (512)
TC = T // NCORES  # steps per core
SLAB2 = TC // 2 + K // 2
SLAB4 = TC // 4 + K // 4
NC_CONST = 13

# fixed model constants (match reference.py)
A, B, C_, D, E, F_, G, H = 0.5616, 0.3126, 48.43, 0.507, 55.0, 0.1538, 90.0, 0.16

# chunking of the half grid
CH = [(0, 296), (296, 592)]

_cache = {}


def _build_nc():
    if "nc" in _cache:
        return _cache["nc"]
    from contextlib import ExitStack
    import concourse.bacc as bacc
    import concourse.tile as tile
    import concourse.mybir as mybir
    from bass_rust import AP

    f32 = mybir.dt.float32
    f16 = mybir.dt.float16
    op = mybir.AluOpType
    ident = mybir.ActivationFunctionType.Identity
    nc = bacc.Bacc("TRN2", target_bir_lowering=False, debug=False,
                   enable_asserts=True, num_devices=NCORES)

    # DRAM I/O
    d_u0e = nc.dram_tensor("u0e", [SLAB2, 1], f16, kind="ExternalInput").ap()
    d_u0o = nc.dram_tensor("u0o", [SLAB2, 1], f16, kind="ExternalInput").ap()
    d_u1e = nc.dram_tensor("u1e", [SLAB2, 1], f16, kind="ExternalInput").ap()
    d_u1o = nc.dram_tensor("u1o", [SLAB2, 1], f16, kind="ExternalInput").ap()
    d_A4 = nc.dram_tensor("A4", [SLAB4, 1], f32, kind="ExternalInput").ap()
    d_B4 = nc.dram_tensor("B4", [SLAB4, 1], f32, kind="ExternalInput").ap()
    d_SA4 = nc.dram_tensor("SA4", [SLAB4, 1], f32, kind="ExternalInput").ap()
    d_gsp = nc.dram_tensor("gsp", [SLAB4, 1], f32, kind="ExternalInput").ap()
    d_Qc = nc.dram_tensor("Qc", [SLAB4, 1], f32, kind="ExternalInput").ap()
    cons = nc.dram_tensor("cons", [P, NC_CONST], f32, kind="ExternalInput").ap()
    iden = nc.dram_tensor("iden", [P, P], f16, kind="ExternalInput").ap()
    nide = nc.dram_tensor("nide", [P, P], f16, kind="ExternalInput").ap()
    o0e = nc.dram_tensor("o0e", [P, LH], f16, kind="ExternalOutput").ap()
    o0o = nc.dram_tensor("o0o", [P, LH], f16, kind="ExternalOutput").ap()
    o1e = nc.dram_tensor("o1e", [P, LH], f16, kind="ExternalOutput").ap()
    o1o = nc.dram_tensor("o1o", [P, LH], f16, kind="ExternalOutput").ap()

    with tile.TileContext(nc) as tc, ExitStack() as ctx:
        pool = ctx.enter_context(tc.tile_pool(name="main", bufs=1))
        ppool = ctx.enter_context(tc.tile_pool(name="psum", bufs=1, space="PSUM"))

        t_u0e = pool.tile([P, W2], f16, name="u0e", tag="u0e")
        t_u0o = pool.tile([P, W2], f16, name="u0o", tag="u0o")
        t_u1e = pool.tile([P, W2], f16, name="u1e", tag="u1e")
        t_u1o = pool.tile([P, W2], f16, name="u1o", tag="u1o")
        t_A4 = pool.tile([P, WC], f32, name="A4", tag="A4")
        t_B4 = pool.tile([P, WC], f32, name="B4", tag="B4")
        t_SA4 = pool.tile([P, WC], f32, name="SA4", tag="SA4")
        t_gsp = pool.tile([P, WC], f32, name="gsp", tag="gsp")
        t_Qc = pool.tile([P, WC], f32, name="Qc", tag="Qc")
        t_cons = pool.tile([P, NC_CONST], f32, name="cons", tag="cons")
        t_iden = pool.tile([P, P], f16, name="iden", tag="iden")
        t_nide = pool.tile([P, P], f16, name="nide", tag="nide")
        t_scr = pool.tile([P, 8], f32, name="scr", tag="scr")

        t_dene = pool.tile([P, W2], f32, name="dene", tag="dene")
        t_deno = pool.tile([P, W2], f32, name="deno", tag="deno")
        t_rece = pool.tile([P, W2], f32, name="rece", tag="rece")
        t_reco = pool.tile([P, W2], f32, name="reco", tag="reco")
        t_r16e = pool.tile([P, W2], f16, name="r16e", tag="r16e")
        t_r16o = pool.tile([P, W2], f16, name="r16o", tag="r16o")
        t_SAe = pool.tile([P, W2], f32, name="SAe", tag="SAe")
        t_SAo = pool.tile([P, W2], f32, name="SAo", tag="SAo")
        t_SBpe = pool.tile([P, W2], f16, name="SBpe", tag="SBpe")
        t_SBpo = pool.tile([P, W2], f16, name="SBpo", tag="SBpo")
        t_SCe = pool.tile([P, W2], f32, name="SCe", tag="SCe")
        t_SCo = pool.tile([P, W2], f32, name="SCo", tag="SCo")
        t_SA2 = pool.tile([P, W2], f32, name="SA2", tag="SA2")

        t_Y0c = pool.tile([P, WC], f32, name="Y0c", tag="Y0c")
        t_c1c = pool.tile([P, WC], f32, name="c1c", tag="c1c")
        t_Y1c = pool.tile([P, WC], f32, name="Y1c", tag="Y1c")
        t_wc = pool.tile([P, WC], f32, name="wc", tag="wc")

        t_a2e = pool.tile([P, W2], f32, name="a2e", tag="a2e")
        t_a2o = pool.tile([P, W2], f32, name="a2o", tag="a2o")
        t_Ad2 = pool.tile([P, W2], f32, name="Ad2", tag="Ad2")
        t_Bd2 = pool.tile([P, W2], f32, name="Bd2", tag="Bd2")
        t_M1 = pool.tile([P, W2], f16, name="M1", tag="M1")
        t_M2 = pool.tile([P, W2], f32, name="M2", tag="M2")
        t_Y0be = pool.tile([P, W2], f16, name="Y0be", tag="Y0be")
        t_Y0bo = pool.tile([P, W2], f16, name="Y0bo", tag="Y0bo")
        t_Y1be = pool.tile([P, W2], f16, name="Y1be", tag="Y1be")
        t_Y1bo = pool.tile([P, W2], f16, name="Y1bo", tag="Y1bo")

        # PSUM: one tile per chunk, each within a single 2KB bank
        p_c2e = [ppool.tile([P, hi - lo], f32, name=f"c2e{d}", tag=f"c2e{d}")
                 for d, (lo, hi) in enumerate(CH)]
        p_bd = [ppool.tile([P, hi - lo], f32, name=f"bd{d}", tag=f"bd{d}")
                for d, (lo, hi) in enumerate(CH)]

        def cst(i):
            return t_cons[:, i : i + 1]

        # ---- preamble: engine warms + DMA issue --------------------------
        nc.gpsimd.memset(t_scr[:, 0:4], 0.0)
        nc.scalar.activation(t_scr[:, 0:1], t_scr[:, 1:2], ident,
                             bias=0.0, scale=1.0)
        nc.scalar.dma_start(t_cons[:], cons[:])

        def dma_plane(eng, dst, src, stride, n, half):
            off = half * 64 * stride
            win = AP(src.tensor, off, [[stride, 64], [1, n]])
            eng.dma_start(dst[64 * half : 64 * (half + 1), :], win)

        # coarse package first (feeds the DVE scan chain)
        dma_plane(nc.sync, t_A4, d_A4, L // 4, WC, 0)
        dma_plane(nc.sync, t_A4, d_A4, L // 4, WC, 1)
        dma_plane(nc.scalar, t_B4, d_B4, L // 4, WC, 0)
        dma_plane(nc.scalar, t_B4, d_B4, L // 4, WC, 1)
        dma_plane(nc.gpsimd, t_SA4, d_SA4, L // 4, WC, 0)
        dma_plane(nc.gpsimd, t_SA4, d_SA4, L // 4, WC, 1)
        dma_plane(nc.sync, t_u1e, d_u1e, L // 2, W2, 0)
        dma_plane(nc.sync, t_u1e, d_u1e, L // 2, W2, 1)
        dma_plane(nc.scalar, t_u1o, d_u1o, L // 2, W2, 0)
        dma_plane(nc.scalar, t_u1o, d_u1o, L // 2, W2, 1)
        dma_plane(nc.gpsimd, t_gsp, d_gsp, L // 4, WC, 0)
        dma_plane(nc.gpsimd, t_gsp, d_gsp, L // 4, WC, 1)
        dma_plane(nc.sync, t_u0e, d_u0e, L // 2, W2, 0)
        dma_plane(nc.sync, t_u0e, d_u0e, L // 2, W2, 1)
        dma_plane(nc.scalar, t_u0o, d_u0o, L // 2, W2, 0)
        dma_plane(nc.scalar, t_u0o, d_u0o, L // 2, W2, 1)
        dma_plane(nc.gpsimd, t_Qc, d_Qc, L // 4, WC, 0)
        dma_plane(nc.gpsimd, t_Qc, d_Qc, L // 4, WC, 1)
        nc.gpsimd.dma_start(t_iden[:], iden[:])
        nc.gpsimd.dma_start(t_nide[:], nide[:])

        # scan column-0 inits
        nc.scalar.activation(t_Y0c[:, 0:1], cst(10), ident, bias=0.0, scale=1.0)
        nc.scalar.activation(t_Y1c[:, 0:1], cst(12), ident, bias=0.0, scale=1.0)
        nc.scalar.activation(t_Y0be[:, 0:1], cst(10), ident, bias=0.0, scale=1.0)
        nc.scalar.activation(t_Y1be[:, 0:1], cst(11), ident, bias=0.0, scale=1.0)

        # ---- op builders (full-width precompute) -------------------------
        def den_(which):
            t_u, t_den = (t_u1e, t_dene) if which == "e" else (t_u1o, t_deno)
            nc.gpsimd.tensor_scalar(t_den[:], t_u[:], cst(0), cst(1),
                                    op.mult, op.add)

        def rec_(which):
            t_den, t_rec = (t_dene, t_rece) if which == "e" else (t_deno, t_reco)
            nc.vector.reciprocal_approx_fast(t_rec[:], t_den[:])

        def r16_(which):
            t_rec, t_r16 = (t_rece, t_r16e) if which == "e" else (t_reco, t_r16o)
            nc.scalar.activation(t_r16[:], t_rec[:], ident, bias=0.0, scale=1.0)

        def SA_(which):
            t_rec, t_SA = (t_rece, t_SAe) if which == "e" else (t_reco, t_SAo)
            nc.scalar.activation(t_SA[:], t_rec[:], ident,
                                 bias=cst(3), scale=cst(2))

        def SBp_(which):
            if which == "e":
                nc.scalar.activation(t_SBpe[:], t_u0e[:], ident,
                                     bias=cst(5), scale=cst(4))
            else:
                nc.scalar.activation(t_SBpo[:], t_u0o[:], ident,
                                     bias=cst(6), scale=cst(4))

        def SC_(which):
            t_u, t_SC = (t_u0e, t_SCe) if which == "e" else (t_u0o, t_SCo)
            nc.gpsimd.tensor_scalar(t_SC[:], t_u[:], cst(7), cst(8),
                                    op.mult, op.add)

        def scanY0c():
            nc.vector.tensor_tensor_scan(t_Y0c[:, 1:WC], t_A4[:, 0:WC-1],
                                         t_B4[:, 0:WC-1], cst(10),
                                         op.mult, op.add)

        def c1c_():
            nc.vector.tensor_tensor(t_c1c[:], t_gsp[:], t_Y0c[:], op.mult)
            nc.vector.tensor_tensor(t_c1c[:], t_c1c[:], t_Qc[:], op.add)

        def scanY1c():
            nc.vector.tensor_tensor_scan(t_Y1c[:, 1:WC], t_SA4[:, 0:WC-1],
                                         t_c1c[:, 0:WC-1], cst(12),
                                         op.mult, op.add)

        def wc_():
            nc.vector.scalar_tensor_tensor(t_wc[:], t_Y0c[:], cst(9),
                                           t_Y1c[:], op.mult, op.add)

        def wc_view(d):
            # broadcast each w_c col to 2 half-grid cols (stride-0 inner dim)
            lo, hi = CH[d]
            n = (hi - lo) // 2
            return t_wc[:, lo // 2 : lo // 2 + n].unsqueeze(2).broadcast_to([P, n, 2])

        def a2_(d, which):
            lo, hi = CH[d]
            t_SC, t_a2 = (t_SCe, t_a2e) if which == "e" else (t_SCo, t_a2o)
            nc.vector.tensor_tensor(t_a2[:, lo:hi], wc_view(d),
                                    t_SC[:, lo:hi], op.add)

        def Ad2_(d):
            lo, hi = CH[d]
            nc.vector.tensor_tensor(t_Ad2[:, lo:hi], t_a2e[:, lo:hi],
                                    t_a2o[:, lo:hi], op.mult)

        def Bd2_(d):
            lo, hi = CH[d]
            nc.scalar.activation(t_Bd2[:, lo:hi], t_a2o[:, lo:hi], ident,
                                 bias=1.0, scale=1.0)

        def SA2_(d):
            lo, hi = CH[d]
            nc.vector.tensor_tensor(t_SA2[:, lo:hi], t_SAe[:, lo:hi],
                                    t_SAo[:, lo:hi], op.mult)

        def scanY0b(d):
            lo, hi = CH[d]
            init = cst(10) if d == 0 else t_Y0be[:, lo : lo + 1]
            n = hi - 1 if d == len(CH) - 1 else hi
            nc.vector.tensor_tensor_scan(t_Y0be[:, lo + 1 : n + 1],
                                         t_Ad2[:, lo:n], t_Bd2[:, lo:n],
                                         init, op.mult, op.add)

        def Y0bo_(d):
            lo, hi = CH[d]
            nc.gpsimd.tensor_tensor(t_Y0bo[:, lo:hi], t_a2e[:, lo:hi],
                                    t_Y0be[:, lo:hi], op.mult)

        def c2e_(d):
            lo, hi = CH[d]
            tp = p_c2e[d][:]
            nc.tensor.matmul(tp, t_iden[:], t_Y0be[:, lo:hi],
                             start=True, stop=False)
            nc.tensor.matmul(tp, t_iden[:], t_SBpe[:, lo:hi],
                             start=False, stop=False)
            nc.tensor.matmul(tp, t_nide[:], t_r16e[:, lo:hi],
                             start=False, stop=True)

        def M1_(d):
            lo, hi = CH[d]
            nc.vector.tensor_tensor(t_M1[:, lo:hi], t_SAo[:, lo:hi],
                                    p_c2e[d][:], op.mult)

        def bd_(d):
            lo, hi = CH[d]
            tp = p_bd[d][:]
            nc.tensor.matmul(tp, t_iden[:], t_M1[:, lo:hi],
                             start=True, stop=False)
            nc.tensor.matmul(tp, t_iden[:], t_Y0bo[:, lo:hi],
                             start=False, stop=False)
            nc.tensor.matmul(tp, t_iden[:], t_SBpo[:, lo:hi],
                             start=False, stop=False)
            nc.tensor.matmul(tp, t_nide[:], t_r16o[:, lo:hi],
                             start=False, stop=True)

        def scanY1b(d):
            lo, hi = CH[d]
            init = cst(11) if d == 0 else t_Y1be[:, lo : lo + 1]
            n = hi - 1 if d == len(CH) - 1 else hi
            nc.vector.tensor_tensor_scan(t_Y1be[:, lo + 1 : n + 1],
                                         t_SA2[:, lo:n], p_bd[d][:, 0 : n - lo],
                                         init, op.mult, op.add)

        def M2_(d):
            lo, hi = CH[d]
            nc.vector.tensor_tensor(t_M2[:, lo:hi], t_SAe[:, lo:hi],
                                    t_Y1be[:, lo:hi], op.mult)

        def Y1bo_(d):
            lo, hi = CH[d]
            nc.vector.tensor_tensor(t_Y1bo[:, lo:hi], t_M2[:, lo:hi],
                                    p_c2e[d][:], op.add)

        def outs(d):
            lo, hi = CH[d]
            olo, ohi = max(lo, GO) - GO, hi - GO
            slo = olo + GO
            nc.sync.dma_start(o0e[:, olo:ohi], t_Y0be[:, slo:hi])
            nc.scalar.dma_start(o0o[:, olo:ohi], t_Y0bo[:, slo:hi])
            nc.sync.dma_start(o1e[:, olo:ohi], t_Y1be[:, slo:hi])
            nc.scalar.dma_start(o1o[:, olo:ohi], t_Y1bo[:, slo:hi])

        # ---- pipelined emission ------------------------------------------
        den_("e"); den_("o")          # gp
        scanY0c()                     # DVE
        SC_("e"); SC_("o")            # gp
        c1c_()                        # DVE
        rec_("e")                     # DVE
        scanY1c()                     # DVE
        SA_("e")                      # ACT (after rec_e)
        rec_("o")                     # DVE
        wc_()                         # DVE
        SA_("o"); r16_("e"); r16_("o")  # ACT
        a2_(0, "e"); a2_(0, "o")      # DVE
        SBp_("e"); SBp_("o")          # ACT
        Ad2_(0)                       # DVE
        Bd2_(0)                       # ACT
        SA2_(0)                       # DVE
        scanY0b(0)                    # DVE
        a2_(1, "e"); a2_(1, "o")      # DVE
        Y0bo_(0)                      # gp
        c2e_(0)                       # PE
        Ad2_(1)                       # DVE
        Bd2_(1)                       # ACT
        M1_(0)                        # DVE
        SA2_(1)                       # DVE
        bd_(0)                        # PE
        scanY0b(1)                    # DVE
        Y0bo_(1)                      # gp
        c2e_(1)                       # PE
        scanY1b(0)                    # DVE
        M1_(1)                        # DVE
        M2_(0)                        # DVE
        bd_(1)                        # PE
        Y1bo_(0)                      # DVE
        outs(0)
        scanY1b(1)                    # DVE
        M2_(1)                        # DVE
        Y1bo_(1)                      # DVE
        outs(1)

    nc.compile()
    _cache["nc"] = nc
    return nc


def _derive(params, x0):
    M, Cc, UA2, Cp, lam, lams, F1, X1p, F3, T1, T200 = [float(params[i]) for i in range(11)]
    UA1 = H * (F1 + F3)
    k1 = (UA1 + F1 * Cp) / lam
    p_ = k1 * B
    q_ = k1 * A
    alpha_u = UA1 * F_ / lam
    alpha_c = (UA1 * G + F1 * Cp * T1) / lam - k1 * C_
    c01 = F1 * X1p / M
    c02 = p_ / M
    c03 = q_ / M
    a10 = -p_ / Cc
    cA2 = -D / (lam * Cc)
    cA1 = 1.0 - q_ / Cc
    cB2 = alpha_u / Cc
    cB1 = alpha_c / Cc
    cB3 = -(E - T200) / (lam * Cc)
    cC2 = alpha_u / M
    cC1 = 1.0 - (F1 - alpha_c) / M
    i0, i1 = float(x0[0]), float(x0[1])
    al = a10 * c01                 # alpha (< 0)
    s_ = -cB3 * UA2 * UA2          # > 0

    cv = np.zeros(17, np.float64)
    cv[0] = cC2                           # a1 scale
    cv[1] = cC1 - (c02 * i0 + c03 * i1)   # a1 bias
    cv[2] = 2.0 * Cp * al / s_            # den scale (negative)
    cv[3] = UA2 * al / s_                 # den bias (negative)
    cv[4] = -cA2 * UA2 * UA2 * al / s_    # SA scale (of rec)
    cv[5] = cA1 + cA2 * UA2               # SA bias
    cv[6] = cC2                           # SC scale
    cv[7] = cC1                           # SC bias
    cv[8] = cB2 / al                      # SBpa scale
    cv[9] = (cB1 + cB3 * UA2) / al        # SBpa bias
    cv[13] = -c02 * c01                   # w scalar (Y0 coeff)
    cv[14] = -c03 * al                    # Y1 coeff (folded into gs)
    cv[15] = i0 / c01
    cv[16] = i1 / al
    return cv, np.float32(c01), np.float32(al)


def _device_cons(cv):
    c = np.zeros(NC_CONST, np.float64)
    c[0] = -cv[2]           # den scale (positive-den variant)
    c[1] = -cv[3]           # den bias
    c[2] = -cv[4]           # SA scale (of positive rec')
    c[3] = cv[5]            # SA bias
    c[4] = cv[8]            # SBpa scale
    c[5] = cv[9]            # SBpa_e bias
    c[6] = cv[9] + 1.0      # SBpa_o bias (+1 fold)
    c[7] = cv[6]            # SC scale
    c[8] = cv[7]            # SC bias
    c[9] = cv[13]           # w_c scalar
    c[10] = cv[15]          # Y0 init
    c[11] = cv[16]          # Y1b init
    c[12] = cv[16] * cv[14] # Y1c init (scaled)
    return c.astype(np.float32)


def _make_in_maps(u, cv):
    f = np.float32
    uq = np.ascontiguousarray(u, f).astype(np.float16)
    # padded (K leading repeat rows) fp32 view for package composition
    up = np.concatenate([np.repeat(uq[0:1], K, axis=0), uq], axis=0).astype(f)

    a1 = (f(cv[0]) * up[:, 0] + f(cv[1])).astype(f)
    den = (f(cv[2]) * up[:, 1] + f(cv[3])).astype(f)
    rec = (1.0 / den).astype(f)
    SA = (f(cv[4]) * rec + f(cv[5])).astype(f)
    SBr = (f(cv[8]) * up[:, 0] + f(cv[9]) + rec).astype(f)

    # coarse composition (b=1 for the a1 scan)
    A2 = (a1[0::2] * a1[1::2]).astype(f)
    B2 = (a1[1::2] + 1.0).astype(f)
    A4 = (A2[0::2] * A2[1::2]).astype(f)
    B4 = (A2[1::2] * B2[0::2] + B2[1::2]).astype(f)
    SA4 = (SA[0::4] * SA[1::4] * SA[2::4] * SA[3::4]).astype(f)
    SAc = SA[0::4]
    gs = (1.0 + SAc * (1.0 + SAc * (1.0 + SAc))).astype(f)
    gsp = (f(cv[14]) * gs).astype(f)
    Qc = (gsp * SBr[0::4]).astype(f)

    u0e = up[0::2, 0].astype(np.float16)
    u0o = up[1::2, 0].astype(np.float16)
    u1e = up[0::2, 1].astype(np.float16)
    u1o = up[1::2, 1].astype(np.float16)

    cons = np.tile(_device_cons(cv)[None, :], (P, 1))
    eye = np.eye(P, dtype=np.float16)
    neye = -eye

    in_maps = []
    for c in range(NCORES):
        r2 = c * TC // 2
        r4 = c * TC // 4
        in_maps.append({
            "u0e": np.ascontiguousarray(u0e[r2 : r2 + SLAB2])[:, None],
            "u0o": np.ascontiguousarray(u0o[r2 : r2 + SLAB2])[:, None],
            "u1e": np.ascontiguousarray(u1e[r2 : r2 + SLAB2])[:, None],
            "u1o": np.ascontiguousarray(u1o[r2 : r2 + SLAB2])[:, None],
            "A4": np.ascontiguousarray(A4[r4 : r4 + SLAB4])[:, None],
            "B4": np.ascontiguousarray(B4[r4 : r4 + SLAB4])[:, None],
            "SA4": np.ascontiguousarray(SA4[r4 : r4 + SLAB4])[:, None],
            "gsp": np.ascontiguousarray(gsp[r4 : r4 + SLAB4])[:, None],
            "Qc": np.ascontiguousarray(Qc[r4 : r4 + SLAB4])[:, None],
            "cons": cons,
            "iden": eye,
            "nide": neye,
        })
    return in_maps


def _host_head(u, x0, params, n):
    # exact fp32 simulation of the first n steps (window 0 has no spin-up)
    f = np.float32
    M, Cc, UA2, Cp, lam, lams, F1, X1p, F3, T1, T200 = [f(params[i]) for i in range(11)]
    out = np.empty((n, 2), f)
    s0, s1 = f(x0[0]), f(x0[1])
    fA, fB, fC, fD, fE, fF, fG, fH = f(A), f(B), f(C_), f(D), f(E), f(F_), f(G), f(H)
    one, two = f(1.0), f(2.0)
    UA1 = fH * (F1 + F3)
    for t in range(n):
        out[t, 0] = s0
        out[t, 1] = s1
        u0, u1 = f(u[t, 0]), f(u[t, 1])
        T2 = fA * s1 + fB * s0 + fC
        T3 = fD * s1 + fE
        T100 = fF * u0 + fG
        Q100 = UA1 * (T100 - T2)
        Q200 = UA2 * (T3 - T200) / (one + UA2 / (two * Cp * u1))
        F5 = Q200 / lam
        F4 = (Q100 - F1 * Cp * (T2 - T1)) / lam
        F2 = F1 - F4
        X2d = (F1 * X1p - F2 * s0) / M
        P2d = (F4 - F5) / Cc
        s0 = s0 + X2d
        s1 = s1 + P2d
    return out


def _assemble(results, head, c01, al):
    out = np.empty((T, 2), np.float32)
    e0 = np.stack([r["o0e"] for r in results]).astype(np.float32)
    o0 = np.stack([r["o0o"] for r in results]).astype(np.float32)
    e1 = np.stack([r["o1e"] for r in results]).astype(np.float32)
    o1 = np.stack([r["o1o"] for r in results]).astype(np.float32)
    x0v = np.empty((T, 2), np.float32)
    x0v[0::2, 0] = e0.reshape(-1)
    x0v[1::2, 0] = o0.reshape(-1) + 1.0
    x0v[0::2, 1] = e1.reshape(-1)
    x0v[1::2, 1] = o1.reshape(-1)
    out[:, 0] = x0v[:, 0] * c01
    out[:, 1] = x0v[:, 1] * al
    out[0:L] = head
    return out


def run(u_forced, x0, params, trace=False):
    from concourse.bass_utils import run_bass_kernel_spmd
    nc = _build_nc()
    cv, c01, al = _derive(params, x0)
    in_maps = _make_in_maps(u_forced, cv)
    head = _host_head(u_forced, x0, params, L)
    res = run_bass_kernel_spmd(nc, in_maps, list(range(NCORES)), trace=trace)
    return _assemble(res.results, head, c01, al), res


def kernel(u_forced, x0, params):
    out, _ = run(u_forced, x0, params, trace=False)
    return out
